# revision 27
# baseline (speedup 1.0000x reference)
"""GNN ensemble MoE-routing kernel for Trainium2 (8 NeuronCores).

Reference computes all 8 expert MLPs for every sample then selects one
(8x wasted FLOPs). This kernel routes on the host instead: samples are
gathered per expert and core c runs ONLY expert c's MLP (expert-parallel
sharding), with both matmuls in fp8-e4m3 DoubleRow mode (2 contraction
rows per PE cycle -> 2x the fp32r/bf16 matmul throughput).

Math folding (exact):
  lat = eps*sigma_c + mu_c  =>  lat @ W1_c = eps @ (sigma_c*W1_c) + mu_c@W1_c
so mm1 computes  z = eps @ W1p + b1p  with W1p = sigma_c*W1_c folded on
the host.  To keep fp8 quantization error off the large DC component of
the output, the sigmoid is rewritten through tanh:
  sigmoid(z) = 0.5 + 0.5*tanh(z/2)
  y = sigmoid(z) @ W2 + b2 = tanh(z/2) @ (W2/2) + (0.5*sum_k W2[k,:] + b2)
The exact constant term (which carries most of y's variance) is added in
fp32; only the zero-mean tanh term goes through the fp8 matmul.
Quantization (host, max-scaled to 235 < TRN e4m3 max 240):
  epsq = e4m3(eps*s_e), W1q = e4m3(W1p*s_w1), W2q = e4m3(0.5*W2*s_w2)
  t = tanh(psum1 * 0.5/(s_e*s_w1) + b1p/2)   (ScalarE, fp8 out)
  y = psum2 * (1/s_w2) + (b2 + 0.5*sum W2)   (DVE tensor_scalar)
Measured in numpy: rms rel err ~1.6e-2 (threshold 2e-2); mm1-in-f32r
fallback mode ~1.1e-2.

Device layout: features on SBUF partitions, samples on the free axis,
k-subtiles stacked 3D [128, ks, n] so DoubleRow can take ks-pairs.  All
weight/input tensors are pre-packed on the host into their exact SBUF
layout so every dma_start is one dense descriptor (each costs ~0.65us
serialized on the Sync sequencer), ordered on the single fast Sync
HWDGE queue in the order the compute stream consumes them.  The first
chunk runs its mm1 kd-outer so matmuls start after only the front
halves of w1/x0 have landed; outputs store per-oc quarter so the drain
after the last matmul is short.  Dummy fp32 warmup matmuls run during
the initial DMA wait so the PE HAM clock-gate is at 2.4 GHz when real
work arrives.
"""

from contextlib import ExitStack

import ml_dtypes
import numpy as np

import concourse.tile as tile
from concourse import bacc, mybir
from concourse.bass_utils import run_bass_kernel_spmd

NB_COMP = 8
LAT_DIM = 512
NB_NEUR = 1024
OUT_DIM = 512
N_CORES = 8

F32 = mybir.dt.float32
F32R = mybir.dt.float32r
BF16 = mybir.dt.bfloat16
FP8 = mybir.dt.float8e4
E4M3 = ml_dtypes.float8_e4m3
TANH = mybir.ActivationFunctionType.Tanh
DR = mybir.MatmulPerfMode.DoubleRow
MULT = mybir.AluOpType.mult
ADD = mybir.AluOpType.add

KS1 = LAT_DIM // 128  # 4 k-subtiles for mm1
KS2 = NB_NEUR // 128  # 8 k-subtiles for mm2
MC1 = NB_NEUR // 128  # 8 output tiles for mm1
MC2 = OUT_DIM // 128  # 4 output tiles for mm2

QMAX = 235.0  # stay under TRN e4m3 max-normal 240
N_WARM = 8

_program_cache = {}


def _make_chunks(k_cap):
    # First chunk 256-wide (fewer bytes gate the pipeline start), big
    # chunks after, small remainder LAST so the store drain is minimal.
    sizes = [min(256, k_cap)]
    left = k_cap - sizes[0]
    while left > 512:
        sizes.append(512)
        left -= 512
    if left > 256:
        sizes.append(256)
        left -= 256
    if left > 0:
        sizes.append(left)
    chunks = []
    n0 = 0
    for ns in sizes:
        chunks.append((n0, ns))
        n0 += ns
    return chunks


def _build_program(k_cap, s1_imm, s2_imm):
    """One-expert fp8-DoubleRow MLP over k_cap samples, SPMD on 8 cores."""
    chunks = _make_chunks(k_cap)

    nc = bacc.Bacc(
        "TRN2",
        target_bir_lowering=False,
        debug=False,
        enable_asserts=False,
        num_devices=N_CORES,
    )
    # chunk-blocked: chunk c occupies contiguous cols [KS1*n0, KS1*(n0+ns))
    # so every chunk load is one dense multi-KB-per-partition run
    xq = nc.dram_tensor("xq", [128, KS1 * k_cap], FP8, kind="ExternalInput").ap()
    w1 = nc.dram_tensor("w1", [128, KS1, NB_NEUR], FP8, kind="ExternalInput").ap()
    b1 = nc.dram_tensor("b1", [128, MC1], F32, kind="ExternalInput").ap()
    w2 = nc.dram_tensor("w2", [128, KS2, OUT_DIM], FP8, kind="ExternalInput").ap()
    cb2 = nc.dram_tensor("cb2", [128, MC2], F32, kind="ExternalInput").ap()
    yT = nc.dram_tensor("yT", [128, MC2 * k_cap], BF16, kind="ExternalOutput").ap()

    with tile.TileContext(nc) as tc, ExitStack() as ctx:
        wpool = ctx.enter_context(tc.tile_pool(name="weights", bufs=1))
        xpool = ctx.enter_context(tc.tile_pool(name="x", bufs=3))
        hpool = ctx.enter_context(tc.tile_pool(name="h", bufs=2))
        ypool = ctx.enter_context(tc.tile_pool(name="y", bufs=2))
        pspool = ctx.enter_context(tc.tile_pool(name="ps", bufs=8, space="PSUM"))

        # Warmup: PE busy during the initial DMA wait so HAM un-throttles
        # to 2.4 GHz by the time real matmuls start.
        warm = wpool.tile([128, 128], F32, tag="warm")
        nc.vector.memset(warm[:], 0.0)
        wps = pspool.tile([128, 128], F32, tag="ps", name="warmps")
        for _ in range(N_WARM):
            nc.tensor.matmul(wps[:], warm[:], warm[:], start=True, stop=True)

        # Everything rides the single fast Sync HWDGE queue (the Scalar
        # HWDGE path hard-crashes the exec unit on this runtime, and the
        # GpSimd SWDGE queue generates descriptors too slowly).  The queue
        # transfers in order, so each tensor is enqueued just ahead of
        # when the compute stream needs it: w1/x0 front halves first (the
        # kd-outer first chunk starts on those alone), x1 between the two
        # w2 halves.
        w1t = wpool.tile([128, KS1, NB_NEUR], FP8, tag="w1")
        xts = []
        for cj, (n0, ns) in enumerate(chunks):
            xts.append(xpool.tile([128, KS1, ns], FP8, tag="x", name=f"x{cj}"))
        b1t = wpool.tile([128, MC1], F32, tag="b1")
        w2t = wpool.tile([128, KS2, OUT_DIM], FP8, tag="w2")
        cb2t = wpool.tile([128, MC2], F32, tag="cb2")

        n0, ns = chunks[0]
        # tiny transfer first: absorbs the DMA queue spin-up latency so
        # the first weight/x transfers start at full rate
        nc.sync.dma_start(b1t[:], b1[:])
        nc.sync.dma_start(w1t[:, 0:2, :], w1[:, 0:2, :])
        nc.sync.dma_start(xts[0][:, 0:2, :], xq[:, KS1 * n0 : KS1 * n0 + 2 * ns])
        nc.sync.dma_start(w1t[:, 2:4, :], w1[:, 2:4, :])
        nc.sync.dma_start(xts[0][:, 2:4, :], xq[:, KS1 * n0 + 2 * ns : KS1 * n0 + 4 * ns])
        nc.sync.dma_start(w2t[:, 0:4, :], w2[:, 0:4, :])
        if len(chunks) > 1:
            n0, ns = chunks[1]
            nc.sync.dma_start(xts[1][:], xq[:, KS1 * n0 : KS1 * (n0 + ns)])
        nc.sync.dma_start(w2t[:, 4:8, :], w2[:, 4:8, :])
        nc.sync.dma_start(cb2t[:], cb2[:])

        for ci, (n0, ns) in enumerate(chunks):
            xt = xts[ci]
            if ci >= 2:
                nc.sync.dma_start(xt[:], xq[:, KS1 * n0 : KS1 * (n0 + ns)])

            ht = hpool.tile([128, KS2, ns], FP8, tag="h")
            if ci == 0:
                # kd-outer: the first sweep only needs the front half of
                # w1, so matmuls start before w1's back half lands.
                ps1s = [
                    pspool.tile([128, ns], F32, tag="ps", name=f"ps1_{ci}_{mc}")
                    for mc in range(MC1)
                ]
                for kd in range(0, KS1, 2):
                    for mc in range(MC1):
                        nc.tensor.matmul(
                            ps1s[mc][:],
                            w1t[:, kd : kd + 2, mc * 128 : (mc + 1) * 128],
                            xt[:, kd : kd + 2, :],
                            start=(kd == 0),
                            stop=(kd == KS1 - 2),
                            perf_mode=DR,
                        )
                for mc in range(MC1):
                    nc.scalar.activation(
                        ht[:, mc, :],
                        ps1s[mc][:],
                        TANH,
                        bias=b1t[:, mc : mc + 1],
                        scale=s1_imm,
                    )
            else:
                for mc in range(MC1):
                    ps1 = pspool.tile([128, ns], F32, tag="ps", name=f"ps1_{ci}_{mc}")
                    for kd in range(0, KS1, 2):
                        nc.tensor.matmul(
                            ps1[:],
                            w1t[:, kd : kd + 2, mc * 128 : (mc + 1) * 128],
                            xt[:, kd : kd + 2, :],
                            start=(kd == 0),
                            stop=(kd == KS1 - 2),
                            perf_mode=DR,
                        )
                    nc.scalar.activation(
                        ht[:, mc, :],
                        ps1[:],
                        TANH,
                        bias=b1t[:, mc : mc + 1],
                        scale=s1_imm,
                    )

            yt = ypool.tile([128, MC2, ns], BF16, tag="y")
            for oc in range(MC2):
                ps2 = pspool.tile([128, ns], F32, tag="ps", name=f"ps2_{ci}_{oc}")
                for kd in range(0, KS2, 2):
                    nc.tensor.matmul(
                        ps2[:],
                        w2t[:, kd : kd + 2, oc * 128 : (oc + 1) * 128],
                        ht[:, kd : kd + 2, :],
                        start=(kd == 0),
                        stop=(kd == KS2 - 2),
                        perf_mode=DR,
                    )
                nc.vector.tensor_scalar(
                    yt[:, oc, :], ps2[:], s2_imm, cb2t[:, oc : oc + 1], MULT, ADD
                )
                if ns == 512:
                    # per-oc store: starts draining as each quarter lands
                    nc.sync.dma_start(
                        yT[:, MC2 * n0 + oc * ns : MC2 * n0 + (oc + 1) * ns],
                        yt[:, oc, :],
                    )
            if ns < 512:
                nc.sync.dma_start(yT[:, MC2 * n0 : MC2 * (n0 + ns)], yt[:])

    nc.compile()
    return nc


def get_program(k_cap, s1_imm, s2_imm):
    key = (k_cap, float(s1_imm), float(s2_imm))
    if key not in _program_cache:
        _program_cache[key] = _build_program(k_cap, s1_imm, s2_imm)
    return _program_cache[key]


def _softplus(x):
    x = x.astype(np.float64)
    return (np.maximum(x, 0.0) + np.log1p(np.exp(-np.abs(x)))).astype(np.float32)


def _pack_k(a, nsub):
    """[nsub*128, F] -> [128, nsub, F] with (p, ks, f) = a[ks*128+p, f]."""
    f = a.shape[1]
    return np.ascontiguousarray(a.reshape(nsub, 128, f).transpose(1, 0, 2))


def kernel(epsilon, comp_idx, mu, rho, W1, b1, W2, b2, _trace=False):
    epsilon = np.asarray(epsilon, dtype=np.float32)
    comp_idx = np.asarray(comp_idx, dtype=np.int32)
    mu = np.asarray(mu, dtype=np.float32)
    rho = np.asarray(rho, dtype=np.float32)
    W1 = np.asarray(W1, dtype=np.float32)
    b1 = np.asarray(b1, dtype=np.float32)
    W2 = np.asarray(W2, dtype=np.float32)
    b2 = np.asarray(b2, dtype=np.float32)

    n = epsilon.shape[0]
    sigma = _softplus(rho)  # [C]

    sels = [np.nonzero(comp_idx == c)[0] for c in range(NB_COMP)]
    counts = [len(s) for s in sels]
    k_cap = max(512, -(-max(counts) // 16) * 16)
    chunks = _make_chunks(k_cap)

    # Global (core-uniform) quantization scales -> immediates in the
    # single SPMD program.
    W1p = W1 * sigma[:, None, None]  # [C, 512, 1024]
    W2h = 0.5 * W2
    s_e = QMAX / max(np.abs(epsilon).max(), 1e-30)
    s_w1 = QMAX / max(np.abs(W1p).max(), 1e-30)
    s_w2 = QMAX / max(np.abs(W2h).max(), 1e-30)
    s1_imm = float(0.5 / (s_e * s_w1))
    s2_imm = float(1.0 / s_w2)

    nc = get_program(k_cap, s1_imm, s2_imm)

    in_maps = []
    for c in range(NB_COMP):
        sel = sels[c]
        epsT = np.zeros((LAT_DIM, k_cap), dtype=np.float32)
        if len(sel):
            epsT[:, : len(sel)] = epsilon[sel].T * s_e
        b1p = (
            b1[c].astype(np.float64) + mu[c].astype(np.float64) @ W1[c].astype(np.float64)
        ).astype(np.float32)
        cb2 = (
            b2[c].astype(np.float64) + 0.5 * W2[c].astype(np.float64).sum(axis=0)
        ).astype(np.float32)
        in_maps.append(
            {
                "xq": np.concatenate(
                    [
                        _pack_k(epsT, KS1)[:, :, n0 : n0 + ns].reshape(128, KS1 * ns)
                        for n0, ns in chunks
                    ],
                    axis=1,
                ).astype(E4M3),
                "w1": _pack_k(W1p[c] * s_w1, KS1).astype(E4M3),
                "b1": np.ascontiguousarray((0.5 * b1p).reshape(MC1, 128).T),
                "w2": _pack_k(W2h[c] * s_w2, KS2).astype(E4M3),
                "cb2": np.ascontiguousarray(cb2.reshape(MC2, 128).T),
            }
        )

    res = run_bass_kernel_spmd(
        nc,
        in_maps,
        core_ids=list(range(N_CORES)),
        trace=_trace,
        trace_cores=list(range(N_CORES)) if _trace else None,
    )

    out = np.zeros((n, OUT_DIM), dtype=np.float32)
    for c in range(NB_COMP):
        sel = sels[c]
        if len(sel):
            arr = res.results[c]["yT"]  # [128, MC2*k_cap], chunk-blocked
            yTf = np.empty((128, MC2, k_cap), dtype=np.float32)
            for n0, ns in chunks:
                yTf[:, :, n0 : n0 + ns] = arr[
                    :, MC2 * n0 : MC2 * (n0 + ns)
                ].reshape(128, MC2, ns)
            out[sel] = (
                yTf[:, :, : len(sel)].transpose(2, 1, 0).reshape(len(sel), OUT_DIM)
            )
    if _trace:
        return out, res
    return out


# revision 28
# speedup vs baseline: 1.1531x; 1.1531x over previous
"""GNN ensemble MoE-routing kernel for Trainium2 (8 NeuronCores).

Reference computes all 8 expert MLPs for every sample then selects one
(8x wasted FLOPs). This kernel routes on the host instead: samples are
gathered per expert and core c runs ONLY expert c's MLP (expert-parallel
sharding), with both matmuls in fp8-e4m3 DoubleRow mode (2 contraction
rows per PE cycle -> 2x the fp32r/bf16 matmul throughput).

Math folding (exact):
  lat = eps*sigma_c + mu_c  =>  lat @ W1_c = eps @ (sigma_c*W1_c) + mu_c@W1_c
so mm1 computes  z = eps @ W1p + b1p  with W1p = sigma_c*W1_c folded on
the host.  To keep fp8 quantization error off the large DC component of
the output, the sigmoid is rewritten through tanh:
  sigmoid(z) = 0.5 + 0.5*tanh(z/2)
  y = sigmoid(z) @ W2 + b2 = tanh(z/2) @ (W2/2) + (0.5*sum_k W2[k,:] + b2)
The exact constant term (which carries most of y's variance) is added in
fp32; only the zero-mean tanh term goes through the fp8 matmul.
Quantization (host, max-scaled to 235 < TRN e4m3 max 240):
  epsq = e4m3(eps*s_e), W1q = e4m3(W1p*s_w1), W2q = e4m3(0.5*W2*s_w2)
  t = tanh(psum1 * 0.5/(s_e*s_w1) + b1p/2)   (ScalarE, fp8 out)
  y = psum2 * (1/s_w2) + (b2 + 0.5*sum W2)   (DVE tensor_scalar)
Measured in numpy: rms rel err ~1.6e-2 (threshold 2e-2); mm1-in-f32r
fallback mode ~1.1e-2.

Device layout: features on SBUF partitions, samples on the free axis,
k-subtiles stacked 3D [128, ks, n] so DoubleRow can take ks-pairs.  All
weight/input tensors are pre-packed on the host into their exact SBUF
layout so every dma_start is one dense descriptor (each costs ~0.65us
serialized on the Sync sequencer), ordered on the single fast Sync
HWDGE queue in the order the compute stream consumes them.  The first
chunk runs its mm1 kd-outer so matmuls start after only the front
halves of w1/x0 have landed; outputs store per-oc quarter so the drain
after the last matmul is short.  Dummy fp32 warmup matmuls run during
the initial DMA wait so the PE HAM clock-gate is at 2.4 GHz when real
work arrives.
"""

from contextlib import ExitStack

import ml_dtypes
import numpy as np

import concourse.tile as tile
from concourse import bacc, mybir
from concourse.bass_utils import run_bass_kernel_spmd

NB_COMP = 8
LAT_DIM = 512
NB_NEUR = 1024
OUT_DIM = 512
N_CORES = 8

F32 = mybir.dt.float32
F32R = mybir.dt.float32r
BF16 = mybir.dt.bfloat16
FP8 = mybir.dt.float8e4
E4M3 = ml_dtypes.float8_e4m3
TANH = mybir.ActivationFunctionType.Tanh
DR = mybir.MatmulPerfMode.DoubleRow
MULT = mybir.AluOpType.mult
ADD = mybir.AluOpType.add

KS1 = LAT_DIM // 128  # 4 k-subtiles for mm1
KS2 = NB_NEUR // 128  # 8 k-subtiles for mm2
MC1 = NB_NEUR // 128  # 8 output tiles for mm1
MC2 = OUT_DIM // 128  # 4 output tiles for mm2

QMAX = 235.0  # stay under TRN e4m3 max-normal 240
N_WARM = 8

_program_cache = {}


def _make_chunks(k_cap):
    # Big 512-wide chunks first (the minimum chunk count — each extra
    # chunk costs ~1.5us of pipeline slack), small remainder LAST so the
    # post-compute store drain is minimal.
    chunks = []
    n0 = 0
    while n0 < k_cap:
        ns = min(512, k_cap - n0)
        chunks.append((n0, ns))
        n0 += ns
    return chunks


def _build_program(k_cap, s1_imm, s2_imm):
    """One-expert fp8-DoubleRow MLP over k_cap samples, SPMD on 8 cores."""
    chunks = _make_chunks(k_cap)

    nc = bacc.Bacc(
        "TRN2",
        target_bir_lowering=False,
        debug=False,
        enable_asserts=False,
        num_devices=N_CORES,
    )
    # chunk-blocked: chunk c occupies contiguous cols [KS1*n0, KS1*(n0+ns))
    # so every chunk load is one dense multi-KB-per-partition run
    xq = nc.dram_tensor("xq", [128, KS1 * k_cap], FP8, kind="ExternalInput").ap()
    w1 = nc.dram_tensor("w1", [128, KS1, NB_NEUR], FP8, kind="ExternalInput").ap()
    b1 = nc.dram_tensor("b1", [128, MC1], F32, kind="ExternalInput").ap()
    w2 = nc.dram_tensor("w2", [128, KS2, OUT_DIM], FP8, kind="ExternalInput").ap()
    cb2 = nc.dram_tensor("cb2", [128, MC2], F32, kind="ExternalInput").ap()
    yT = nc.dram_tensor("yT", [128, MC2 * k_cap], BF16, kind="ExternalOutput").ap()

    with tile.TileContext(nc) as tc, ExitStack() as ctx:
        wpool = ctx.enter_context(tc.tile_pool(name="weights", bufs=1))
        xpool = ctx.enter_context(tc.tile_pool(name="x", bufs=3))
        hpool = ctx.enter_context(tc.tile_pool(name="h", bufs=2))
        ypool = ctx.enter_context(tc.tile_pool(name="y", bufs=2))
        pspool = ctx.enter_context(tc.tile_pool(name="ps", bufs=8, space="PSUM"))

        # Warmup: PE busy during the initial DMA wait so HAM un-throttles
        # to 2.4 GHz by the time real matmuls start.
        warm = wpool.tile([128, 128], F32, tag="warm")
        nc.vector.memset(warm[:], 0.0)
        wps = pspool.tile([128, 128], F32, tag="ps", name="warmps")
        for _ in range(N_WARM):
            nc.tensor.matmul(wps[:], warm[:], warm[:], start=True, stop=True)

        # Everything rides the single fast Sync HWDGE queue (the Scalar
        # HWDGE path hard-crashes the exec unit on this runtime, and the
        # GpSimd SWDGE queue generates descriptors too slowly).  The queue
        # transfers in order, so each tensor is enqueued just ahead of
        # when the compute stream needs it: w1/x0 front halves first (the
        # kd-outer first chunk starts on those alone), x1 between the two
        # w2 halves.
        w1t = wpool.tile([128, KS1, NB_NEUR], FP8, tag="w1")
        xts = []
        for cj, (n0, ns) in enumerate(chunks):
            xts.append(xpool.tile([128, KS1, ns], FP8, tag="x", name=f"x{cj}"))
        b1t = wpool.tile([128, MC1], F32, tag="b1")
        w2t = wpool.tile([128, KS2, OUT_DIM], FP8, tag="w2")
        cb2t = wpool.tile([128, MC2], F32, tag="cb2")

        n0, ns = chunks[0]
        # tiny transfer first: absorbs the DMA queue spin-up latency so
        # the first weight/x transfers start at full rate
        nc.sync.dma_start(b1t[:], b1[:])
        nc.sync.dma_start(w1t[:, 0:2, :], w1[:, 0:2, :])
        nc.sync.dma_start(xts[0][:, 0:2, :], xq[:, KS1 * n0 : KS1 * n0 + 2 * ns])
        nc.sync.dma_start(w1t[:, 2:4, :], w1[:, 2:4, :])
        nc.sync.dma_start(xts[0][:, 2:4, :], xq[:, KS1 * n0 + 2 * ns : KS1 * n0 + 4 * ns])
        nc.sync.dma_start(w2t[:, 0:4, :], w2[:, 0:4, :])
        if len(chunks) > 1:
            n0, ns = chunks[1]
            nc.sync.dma_start(xts[1][:], xq[:, KS1 * n0 : KS1 * (n0 + ns)])
        nc.sync.dma_start(w2t[:, 4:8, :], w2[:, 4:8, :])
        nc.sync.dma_start(cb2t[:], cb2[:])

        for ci, (n0, ns) in enumerate(chunks):
            xt = xts[ci]
            if ci >= 2:
                nc.sync.dma_start(xt[:], xq[:, KS1 * n0 : KS1 * (n0 + ns)])

            ht = hpool.tile([128, KS2, ns], FP8, tag="h")
            if ci == 0:
                # kd-outer: the first sweep only needs the front half of
                # w1, so matmuls start before w1's back half lands.
                ps1s = [
                    pspool.tile([128, ns], F32, tag="ps", name=f"ps1_{ci}_{mc}")
                    for mc in range(MC1)
                ]
                for kd in range(0, KS1, 2):
                    for mc in range(MC1):
                        nc.tensor.matmul(
                            ps1s[mc][:],
                            w1t[:, kd : kd + 2, mc * 128 : (mc + 1) * 128],
                            xt[:, kd : kd + 2, :],
                            start=(kd == 0),
                            stop=(kd == KS1 - 2),
                            perf_mode=DR,
                        )
                for mc in range(MC1):
                    nc.scalar.activation(
                        ht[:, mc, :],
                        ps1s[mc][:],
                        TANH,
                        bias=b1t[:, mc : mc + 1],
                        scale=s1_imm,
                    )
            else:
                for mc in range(MC1):
                    ps1 = pspool.tile([128, ns], F32, tag="ps", name=f"ps1_{ci}_{mc}")
                    for kd in range(0, KS1, 2):
                        nc.tensor.matmul(
                            ps1[:],
                            w1t[:, kd : kd + 2, mc * 128 : (mc + 1) * 128],
                            xt[:, kd : kd + 2, :],
                            start=(kd == 0),
                            stop=(kd == KS1 - 2),
                            perf_mode=DR,
                        )
                    nc.scalar.activation(
                        ht[:, mc, :],
                        ps1[:],
                        TANH,
                        bias=b1t[:, mc : mc + 1],
                        scale=s1_imm,
                    )

            yt = ypool.tile([128, MC2, ns], BF16, tag="y")
            for oc in range(MC2):
                ps2 = pspool.tile([128, ns], F32, tag="ps", name=f"ps2_{ci}_{oc}")
                for kd in range(0, KS2, 2):
                    nc.tensor.matmul(
                        ps2[:],
                        w2t[:, kd : kd + 2, oc * 128 : (oc + 1) * 128],
                        ht[:, kd : kd + 2, :],
                        start=(kd == 0),
                        stop=(kd == KS2 - 2),
                        perf_mode=DR,
                    )
                nc.vector.tensor_scalar(
                    yt[:, oc, :], ps2[:], s2_imm, cb2t[:, oc : oc + 1], MULT, ADD
                )
                if ns == 512:
                    # per-oc store: starts draining as each quarter lands
                    nc.sync.dma_start(
                        yT[:, MC2 * n0 + oc * ns : MC2 * n0 + (oc + 1) * ns],
                        yt[:, oc, :],
                    )
            if ns < 512:
                nc.sync.dma_start(yT[:, MC2 * n0 : MC2 * (n0 + ns)], yt[:])

    nc.compile()
    return nc


def get_program(k_cap, s1_imm, s2_imm):
    key = (k_cap, float(s1_imm), float(s2_imm))
    if key not in _program_cache:
        _program_cache[key] = _build_program(k_cap, s1_imm, s2_imm)
    return _program_cache[key]


def _softplus(x):
    x = x.astype(np.float64)
    return (np.maximum(x, 0.0) + np.log1p(np.exp(-np.abs(x)))).astype(np.float32)


def _pack_k(a, nsub):
    """[nsub*128, F] -> [128, nsub, F] with (p, ks, f) = a[ks*128+p, f]."""
    f = a.shape[1]
    return np.ascontiguousarray(a.reshape(nsub, 128, f).transpose(1, 0, 2))


def kernel(epsilon, comp_idx, mu, rho, W1, b1, W2, b2, _trace=False):
    epsilon = np.asarray(epsilon, dtype=np.float32)
    comp_idx = np.asarray(comp_idx, dtype=np.int32)
    mu = np.asarray(mu, dtype=np.float32)
    rho = np.asarray(rho, dtype=np.float32)
    W1 = np.asarray(W1, dtype=np.float32)
    b1 = np.asarray(b1, dtype=np.float32)
    W2 = np.asarray(W2, dtype=np.float32)
    b2 = np.asarray(b2, dtype=np.float32)

    n = epsilon.shape[0]
    sigma = _softplus(rho)  # [C]

    sels = [np.nonzero(comp_idx == c)[0] for c in range(NB_COMP)]
    counts = [len(s) for s in sels]
    k_cap = max(512, -(-max(counts) // 16) * 16)
    chunks = _make_chunks(k_cap)

    # Global (core-uniform) quantization scales -> immediates in the
    # single SPMD program.
    W1p = W1 * sigma[:, None, None]  # [C, 512, 1024]
    W2h = 0.5 * W2
    s_e = QMAX / max(np.abs(epsilon).max(), 1e-30)
    s_w1 = QMAX / max(np.abs(W1p).max(), 1e-30)
    s_w2 = QMAX / max(np.abs(W2h).max(), 1e-30)
    s1_imm = float(0.5 / (s_e * s_w1))
    s2_imm = float(1.0 / s_w2)

    nc = get_program(k_cap, s1_imm, s2_imm)

    in_maps = []
    for c in range(NB_COMP):
        sel = sels[c]
        epsT = np.zeros((LAT_DIM, k_cap), dtype=np.float32)
        if len(sel):
            epsT[:, : len(sel)] = epsilon[sel].T * s_e
        b1p = (
            b1[c].astype(np.float64) + mu[c].astype(np.float64) @ W1[c].astype(np.float64)
        ).astype(np.float32)
        cb2 = (
            b2[c].astype(np.float64) + 0.5 * W2[c].astype(np.float64).sum(axis=0)
        ).astype(np.float32)
        in_maps.append(
            {
                "xq": np.concatenate(
                    [
                        _pack_k(epsT, KS1)[:, :, n0 : n0 + ns].reshape(128, KS1 * ns)
                        for n0, ns in chunks
                    ],
                    axis=1,
                ).astype(E4M3),
                "w1": _pack_k(W1p[c] * s_w1, KS1).astype(E4M3),
                "b1": np.ascontiguousarray((0.5 * b1p).reshape(MC1, 128).T),
                "w2": _pack_k(W2h[c] * s_w2, KS2).astype(E4M3),
                "cb2": np.ascontiguousarray(cb2.reshape(MC2, 128).T),
            }
        )

    res = run_bass_kernel_spmd(
        nc,
        in_maps,
        core_ids=list(range(N_CORES)),
        trace=_trace,
        trace_cores=list(range(N_CORES)) if _trace else None,
    )

    out = np.zeros((n, OUT_DIM), dtype=np.float32)
    for c in range(NB_COMP):
        sel = sels[c]
        if len(sel):
            arr = res.results[c]["yT"]  # [128, MC2*k_cap], chunk-blocked
            yTf = np.empty((128, MC2, k_cap), dtype=np.float32)
            for n0, ns in chunks:
                yTf[:, :, n0 : n0 + ns] = arr[
                    :, MC2 * n0 : MC2 * (n0 + ns)
                ].reshape(128, MC2, ns)
            out[sel] = (
                yTf[:, :, : len(sel)].transpose(2, 1, 0).reshape(len(sel), OUT_DIM)
            )
    if _trace:
        return out, res
    return out


# revision 29
# speedup vs baseline: 1.1828x; 1.0257x over previous
"""GNN ensemble MoE-routing kernel for Trainium2 (8 NeuronCores).

Reference computes all 8 expert MLPs for every sample then selects one
(8x wasted FLOPs). This kernel routes on the host instead: samples are
gathered per expert and core c runs ONLY expert c's MLP (expert-parallel
sharding), with both matmuls in fp8-e4m3 DoubleRow mode (2 contraction
rows per PE cycle -> 2x the fp32r/bf16 matmul throughput).

Math folding (exact):
  lat = eps*sigma_c + mu_c  =>  lat @ W1_c = eps @ (sigma_c*W1_c) + mu_c@W1_c
so mm1 computes  z = eps @ W1p + b1p  with W1p = sigma_c*W1_c folded on
the host.  To keep fp8 quantization error off the large DC component of
the output, the sigmoid is rewritten through tanh:
  sigmoid(z) = 0.5 + 0.5*tanh(z/2)
  y = sigmoid(z) @ W2 + b2 = tanh(z/2) @ (W2/2) + (0.5*sum_k W2[k,:] + b2)
The exact constant term (which carries most of y's variance) is added in
fp32; only the zero-mean tanh term goes through the fp8 matmul.
Quantization (host, max-scaled to 235 < TRN e4m3 max 240):
  epsq = e4m3(eps*s_e), W1q = e4m3(W1p*s_w1), W2q = e4m3(0.5*W2*s_w2)
  t = tanh(psum1 * 0.5/(s_e*s_w1) + b1p/2)   (ScalarE, fp8 out)
  y = psum2 * (1/s_w2) + (b2 + 0.5*sum W2)   (DVE tensor_scalar)
Measured in numpy: rms rel err ~1.6e-2 (threshold 2e-2); mm1-in-f32r
fallback mode ~1.1e-2.

Device layout: features on SBUF partitions, samples on the free axis,
k-subtiles stacked 3D [128, ks, n] so DoubleRow can take ks-pairs.  All
weight/input tensors are pre-packed on the host into their exact SBUF
layout so every dma_start is one dense descriptor (each costs ~0.65us
serialized on the Sync sequencer), ordered on the single fast Sync
HWDGE queue in the order the compute stream consumes them.  The first
chunk runs its mm1 kd-outer so matmuls start after only the front
halves of w1/x0 have landed; outputs store per-oc quarter so the drain
after the last matmul is short.  Dummy fp32 warmup matmuls run during
the initial DMA wait so the PE HAM clock-gate is at 2.4 GHz when real
work arrives.
"""

from contextlib import ExitStack

import ml_dtypes
import numpy as np

import concourse.tile as tile
from concourse import bacc, mybir
from concourse.bass_utils import run_bass_kernel_spmd

NB_COMP = 8
LAT_DIM = 512
NB_NEUR = 1024
OUT_DIM = 512
N_CORES = 8

F32 = mybir.dt.float32
F32R = mybir.dt.float32r
BF16 = mybir.dt.bfloat16
FP8 = mybir.dt.float8e4
E4M3 = ml_dtypes.float8_e4m3
TANH = mybir.ActivationFunctionType.Tanh
DR = mybir.MatmulPerfMode.DoubleRow
MULT = mybir.AluOpType.mult
ADD = mybir.AluOpType.add

KS1 = LAT_DIM // 128  # 4 k-subtiles for mm1
KS2 = NB_NEUR // 128  # 8 k-subtiles for mm2
MC1 = NB_NEUR // 128  # 8 output tiles for mm1
MC2 = OUT_DIM // 128  # 4 output tiles for mm2

QMAX = 235.0  # stay under TRN e4m3 max-normal 240
N_WARM = 8

_program_cache = {}


def _make_chunks(k_cap):
    # Big 512-wide chunks first (the minimum chunk count — each extra
    # chunk costs ~1.5us of pipeline slack), small remainder LAST so the
    # post-compute store drain is minimal.
    chunks = []
    n0 = 0
    while n0 < k_cap:
        ns = min(512, k_cap - n0)
        chunks.append((n0, ns))
        n0 += ns
    return chunks


def _build_program(k_cap, s1_imm, s2_imm):
    """One-expert fp8-DoubleRow MLP over k_cap samples, SPMD on 8 cores."""
    chunks = _make_chunks(k_cap)

    nc = bacc.Bacc(
        "TRN2",
        target_bir_lowering=False,
        debug=False,
        enable_asserts=False,
        num_devices=N_CORES,
    )
    # chunk-blocked: chunk c occupies contiguous cols [KS1*n0, KS1*(n0+ns))
    # so every chunk load is one dense multi-KB-per-partition run
    xq = nc.dram_tensor("xq", [128, KS1 * k_cap], FP8, kind="ExternalInput").ap()
    w1 = nc.dram_tensor("w1", [128, KS1, NB_NEUR], FP8, kind="ExternalInput").ap()
    b1 = nc.dram_tensor("b1", [128, MC1], F32, kind="ExternalInput").ap()
    w2 = nc.dram_tensor("w2", [128, KS2, OUT_DIM], FP8, kind="ExternalInput").ap()
    cb2 = nc.dram_tensor("cb2", [128, MC2], F32, kind="ExternalInput").ap()
    yT = nc.dram_tensor("yT", [128, MC2 * k_cap], BF16, kind="ExternalOutput").ap()

    with tile.TileContext(nc) as tc, ExitStack() as ctx:
        wpool = ctx.enter_context(tc.tile_pool(name="weights", bufs=1))
        xpool = ctx.enter_context(tc.tile_pool(name="x", bufs=3))
        hpool = ctx.enter_context(tc.tile_pool(name="h", bufs=2))
        ypool = ctx.enter_context(tc.tile_pool(name="y", bufs=2))
        pspool = ctx.enter_context(tc.tile_pool(name="ps", bufs=8, space="PSUM"))

        # Warmup: PE busy during the initial DMA wait so HAM un-throttles
        # to 2.4 GHz by the time real matmuls start.
        warm = wpool.tile([128, 128], F32, tag="warm")
        nc.vector.memset(warm[:], 0.0)
        wps = pspool.tile([128, 128], F32, tag="ps", name="warmps")
        for _ in range(N_WARM):
            nc.tensor.matmul(wps[:], warm[:], warm[:], start=True, stop=True)

        # Everything rides the single fast Sync HWDGE queue (the Scalar
        # HWDGE path hard-crashes the exec unit on this runtime, and the
        # GpSimd SWDGE queue generates descriptors too slowly).  The queue
        # transfers in order, so each tensor is enqueued just ahead of
        # when the compute stream needs it: w1/x0 front halves first (the
        # kd-outer first chunk starts on those alone), x1 between the two
        # w2 halves.
        w1t = wpool.tile([128, KS1, NB_NEUR], FP8, tag="w1")
        xts = []
        for cj, (n0, ns) in enumerate(chunks):
            xts.append(xpool.tile([128, KS1, ns], FP8, tag="x", name=f"x{cj}"))
        b1t = wpool.tile([128, MC1], F32, tag="b1")
        w2t = wpool.tile([128, KS2, OUT_DIM], FP8, tag="w2")
        cb2t = wpool.tile([128, MC2], F32, tag="cb2")

        n0, ns = chunks[0]
        # tiny transfer first: absorbs the DMA queue spin-up latency so
        # the first weight/x transfers start at full rate
        nc.sync.dma_start(b1t[:], b1[:])
        nc.sync.dma_start(w1t[:, 0:2, :], w1[:, 0:2, :])
        nc.sync.dma_start(xts[0][:, 0:2, :], xq[:, KS1 * n0 : KS1 * n0 + 2 * ns])
        # back halves: x0 first (every kd2 matmul streams all of it), then
        # w1's back half in two slices so the kd2 sweep's first four
        # mc-matmuls start as soon as the first slice lands.
        nc.sync.dma_start(xts[0][:, 2:4, :], xq[:, KS1 * n0 + 2 * ns : KS1 * n0 + 4 * ns])
        nc.sync.dma_start(w1t[:, 2:4, 0:512], w1[:, 2:4, 0:512])
        nc.sync.dma_start(w1t[:, 2:4, 512:1024], w1[:, 2:4, 512:1024])
        nc.sync.dma_start(w2t[:, 0:4, :], w2[:, 0:4, :])
        if len(chunks) > 1:
            n0, ns = chunks[1]
            nc.sync.dma_start(xts[1][:], xq[:, KS1 * n0 : KS1 * (n0 + ns)])
        nc.sync.dma_start(w2t[:, 4:8, :], w2[:, 4:8, :])
        nc.sync.dma_start(cb2t[:], cb2[:])

        for ci, (n0, ns) in enumerate(chunks):
            xt = xts[ci]
            if ci >= 2:
                nc.sync.dma_start(xt[:], xq[:, KS1 * n0 : KS1 * (n0 + ns)])

            ht = hpool.tile([128, KS2, ns], FP8, tag="h")
            if ci == 0:
                # kd-outer: the first sweep only needs the front half of
                # w1, so matmuls start before w1's back half lands.
                ps1s = [
                    pspool.tile([128, ns], F32, tag="ps", name=f"ps1_{ci}_{mc}")
                    for mc in range(MC1)
                ]
                for kd in range(0, KS1, 2):
                    for mc in range(MC1):
                        nc.tensor.matmul(
                            ps1s[mc][:],
                            w1t[:, kd : kd + 2, mc * 128 : (mc + 1) * 128],
                            xt[:, kd : kd + 2, :],
                            start=(kd == 0),
                            stop=(kd == KS1 - 2),
                            perf_mode=DR,
                        )
                for mc in range(MC1):
                    nc.scalar.activation(
                        ht[:, mc, :],
                        ps1s[mc][:],
                        TANH,
                        bias=b1t[:, mc : mc + 1],
                        scale=s1_imm,
                    )
            else:
                for mc in range(MC1):
                    ps1 = pspool.tile([128, ns], F32, tag="ps", name=f"ps1_{ci}_{mc}")
                    for kd in range(0, KS1, 2):
                        nc.tensor.matmul(
                            ps1[:],
                            w1t[:, kd : kd + 2, mc * 128 : (mc + 1) * 128],
                            xt[:, kd : kd + 2, :],
                            start=(kd == 0),
                            stop=(kd == KS1 - 2),
                            perf_mode=DR,
                        )
                    nc.scalar.activation(
                        ht[:, mc, :],
                        ps1[:],
                        TANH,
                        bias=b1t[:, mc : mc + 1],
                        scale=s1_imm,
                    )

            yt = ypool.tile([128, MC2, ns], BF16, tag="y")
            for oc in range(MC2):
                ps2 = pspool.tile([128, ns], F32, tag="ps", name=f"ps2_{ci}_{oc}")
                for kd in range(0, KS2, 2):
                    nc.tensor.matmul(
                        ps2[:],
                        w2t[:, kd : kd + 2, oc * 128 : (oc + 1) * 128],
                        ht[:, kd : kd + 2, :],
                        start=(kd == 0),
                        stop=(kd == KS2 - 2),
                        perf_mode=DR,
                    )
                nc.vector.tensor_scalar(
                    yt[:, oc, :], ps2[:], s2_imm, cb2t[:, oc : oc + 1], MULT, ADD
                )
                if ns == 512:
                    # per-oc store: starts draining as each quarter lands
                    nc.sync.dma_start(
                        yT[:, MC2 * n0 + oc * ns : MC2 * n0 + (oc + 1) * ns],
                        yt[:, oc, :],
                    )
            if ns < 512:
                nc.sync.dma_start(yT[:, MC2 * n0 : MC2 * (n0 + ns)], yt[:])

    nc.compile()
    return nc


def get_program(k_cap, s1_imm, s2_imm):
    key = (k_cap, float(s1_imm), float(s2_imm))
    if key not in _program_cache:
        _program_cache[key] = _build_program(k_cap, s1_imm, s2_imm)
    return _program_cache[key]


def _softplus(x):
    x = x.astype(np.float64)
    return (np.maximum(x, 0.0) + np.log1p(np.exp(-np.abs(x)))).astype(np.float32)


def _pack_k(a, nsub):
    """[nsub*128, F] -> [128, nsub, F] with (p, ks, f) = a[ks*128+p, f]."""
    f = a.shape[1]
    return np.ascontiguousarray(a.reshape(nsub, 128, f).transpose(1, 0, 2))


def kernel(epsilon, comp_idx, mu, rho, W1, b1, W2, b2, _trace=False):
    epsilon = np.asarray(epsilon, dtype=np.float32)
    comp_idx = np.asarray(comp_idx, dtype=np.int32)
    mu = np.asarray(mu, dtype=np.float32)
    rho = np.asarray(rho, dtype=np.float32)
    W1 = np.asarray(W1, dtype=np.float32)
    b1 = np.asarray(b1, dtype=np.float32)
    W2 = np.asarray(W2, dtype=np.float32)
    b2 = np.asarray(b2, dtype=np.float32)

    n = epsilon.shape[0]
    sigma = _softplus(rho)  # [C]

    sels = [np.nonzero(comp_idx == c)[0] for c in range(NB_COMP)]
    counts = [len(s) for s in sels]
    k_cap = max(512, -(-max(counts) // 16) * 16)
    chunks = _make_chunks(k_cap)

    # Global (core-uniform) quantization scales -> immediates in the
    # single SPMD program.
    W1p = W1 * sigma[:, None, None]  # [C, 512, 1024]
    W2h = 0.5 * W2
    s_e = QMAX / max(np.abs(epsilon).max(), 1e-30)
    s_w1 = QMAX / max(np.abs(W1p).max(), 1e-30)
    s_w2 = QMAX / max(np.abs(W2h).max(), 1e-30)
    s1_imm = float(0.5 / (s_e * s_w1))
    s2_imm = float(1.0 / s_w2)

    nc = get_program(k_cap, s1_imm, s2_imm)

    in_maps = []
    for c in range(NB_COMP):
        sel = sels[c]
        epsT = np.zeros((LAT_DIM, k_cap), dtype=np.float32)
        if len(sel):
            epsT[:, : len(sel)] = epsilon[sel].T * s_e
        b1p = (
            b1[c].astype(np.float64) + mu[c].astype(np.float64) @ W1[c].astype(np.float64)
        ).astype(np.float32)
        cb2 = (
            b2[c].astype(np.float64) + 0.5 * W2[c].astype(np.float64).sum(axis=0)
        ).astype(np.float32)
        in_maps.append(
            {
                "xq": np.concatenate(
                    [
                        _pack_k(epsT, KS1)[:, :, n0 : n0 + ns].reshape(128, KS1 * ns)
                        for n0, ns in chunks
                    ],
                    axis=1,
                ).astype(E4M3),
                "w1": _pack_k(W1p[c] * s_w1, KS1).astype(E4M3),
                "b1": np.ascontiguousarray((0.5 * b1p).reshape(MC1, 128).T),
                "w2": _pack_k(W2h[c] * s_w2, KS2).astype(E4M3),
                "cb2": np.ascontiguousarray(cb2.reshape(MC2, 128).T),
            }
        )

    res = run_bass_kernel_spmd(
        nc,
        in_maps,
        core_ids=list(range(N_CORES)),
        trace=_trace,
        trace_cores=list(range(N_CORES)) if _trace else None,
    )

    out = np.zeros((n, OUT_DIM), dtype=np.float32)
    for c in range(NB_COMP):
        sel = sels[c]
        if len(sel):
            arr = res.results[c]["yT"]  # [128, MC2*k_cap], chunk-blocked
            yTf = np.empty((128, MC2, k_cap), dtype=np.float32)
            for n0, ns in chunks:
                yTf[:, :, n0 : n0 + ns] = arr[
                    :, MC2 * n0 : MC2 * (n0 + ns)
                ].reshape(128, MC2, ns)
            out[sel] = (
                yTf[:, :, : len(sel)].transpose(2, 1, 0).reshape(len(sel), OUT_DIM)
            )
    if _trace:
        return out, res
    return out
